# revision 29
# baseline (speedup 1.0000x reference)
"""Trainium2 Bass kernel for nn_GarmentNet_LF (moe_routing).

Strategy (8 NeuronCores, data-parallel over batch):
- MNet (3x Linear+BN+PReLU -> per-sample expert weights) is computed
  REPLICATED on every core over the full batch B=4096 in fp32r (TF32-like)
  so BatchNorm batch statistics need no collectives. Each core's input m0 is
  ROTATED on host so that the core's own 512-sample slice sits at columns
  0:512 (BN stats are permutation-invariant), making slice extraction static.
  MNet linear biases are dropped: training-mode BN directly after a Linear
  absorbs any per-feature constant shift.
- The 4 all-expert dense layers (64 experts) run batch-sharded (512
  samples/core) in bf16. The per-sample expert weighting wn[b,e] is applied
  on the INPUT side: Zs_e = Z * broadcast(wn[:,e]), so the sum over experts
  accumulates for free in PSUM across 64 experts x k-chunks per output bank.
  The expert bias term sum_e wn[b,e] b_e[o] is one extra matmul
  (lhsT=bias[E,O-chunk], rhs=wnT[E,B]) into the same PSUM bank.
- Shared BatchNorm stats for the 3 inner expert layers are combined across
  cores with one small AllGather each (sum & sqsum per feature); a dummy
  AllGather is issued at kernel start to absorb collective init/start skew.
- PReLU+BN apply is a single ScalarE activation (Prelu with per-partition
  scale/bias/alpha) reading PSUM.

Host-side marshalling (numpy): transposes weights to lhsT layout, casts the
expert path to bf16, rotates m0, pre-replicates tiny per-feature BN params.
Device output is feature-major [64, 512] per core; host reassembles.
"""
import numpy as np
import ml_dtypes

import concourse.bacc as bacc
import concourse.mybir as mybir
import concourse.tile as tile
from concourse.bass_utils import run_bass_kernel_spmd

N_CORES = 8
B = 4096
BC = B // N_CORES          # 512 per core
E = 64
EPS = 1e-5

bf16 = mybir.dt.bfloat16
f32 = mybir.dt.float32
f32r = mybir.dt.float32r
AX = mybir.AxisListType
OP = mybir.AluOpType
AF = mybir.ActivationFunctionType

DEBUG = False

_cache = {}


def _bn_apply_params(nc, sb, stat, nfeat_chunks, total_n, g_ap, b_ap, tag,
                     prescaled=False):
    """stat: [P, 2*nfeat_chunks] cols = sums then sqsums (chunk-major).
    If prescaled, stat already holds [mu | e2]."""
    P = stat.shape[0]
    c = nfeat_chunks
    if prescaled:
        me = stat
    else:
        me = sb.tile([P, 2 * c], f32, name=f"me_{tag}")   # [mu | e2]
        nc.vector.tensor_scalar_mul(me[:], stat[:, 0:2 * c], 1.0 / total_n)
    mu = me[:, 0:c]
    mu2 = sb.tile([P, c], f32, name=f"mu2_{tag}")
    nc.vector.tensor_tensor(out=mu2[:], in0=mu, in1=mu, op=OP.mult)
    var = sb.tile([P, c], f32, name=f"var_{tag}")
    nc.vector.tensor_tensor(out=var[:], in0=me[:, c:2 * c], in1=mu2[:],
                            op=OP.subtract)
    nc.vector.tensor_scalar_add(var[:], var[:], EPS)
    std = sb.tile([P, c], f32, name=f"std_{tag}")
    nc.scalar.sqrt(std[:], var[:])
    rstd = sb.tile([P, c], f32, name=f"rstd_{tag}")
    nc.vector.reciprocal(rstd[:], std[:])
    s_ap = sb.tile([P, c], f32, name=f"s_{tag}")
    nc.vector.tensor_tensor(out=s_ap[:], in0=g_ap, in1=rstd[:], op=OP.mult)
    ms = sb.tile([P, c], f32, name=f"ms_{tag}")
    nc.vector.tensor_tensor(out=ms[:], in0=mu, in1=s_ap[:], op=OP.mult)
    t_ap = sb.tile([P, c], f32, name=f"t_{tag}")
    nc.vector.tensor_tensor(out=t_ap[:], in0=b_ap, in1=ms[:], op=OP.subtract)
    return s_ap, t_ap



def _build():
    nc = bacc.Bacc("TRN2", target_bir_lowering=False, debug=False,
                   enable_asserts=False, num_devices=N_CORES)

    # ---------------- DRAM parameters ----------------
    din = {}

    def dr(name, shape, dt):
        din[name] = nc.dram_tensor(name, shape, dt, kind="ExternalInput")
        return din[name]

    dr("m0T", [128, B], f32)            # rotated per core
    dr("x0T", [64, BC], bf16)           # core's slice, feature-major
    dr("mW1T", [128, 256], f32)
    dr("mW2T", [256, 128], f32)
    dr("mW3T", [128, 64], f32)
    dr("mg1c", [128, 2], f32)
    dr("mbe1c", [128, 2], f32)
    dr("ma1r", [128, 1], f32)
    dr("mg2c", [128, 1], f32)
    dr("mbe2c", [128, 1], f32)
    dr("ma2r", [128, 1], f32)
    dr("mg3c", [64, 1], f32)
    dr("mbe3c", [64, 1], f32)
    dr("ma3r", [64, 1], f32)
    dr("bngc", [128, 2], f32)
    dr("bnbc", [128, 2], f32)
    dr("ar", [128, 1], f32)
    Wenc0T = dr("Wenc0T", [E, 64, 256], bf16)
    Wenc1T = dr("Wenc1T", [E, 256, 256], bf16)
    Wdec0T = dr("Wdec0T", [E, 256, 256], bf16)
    Wdec1T = dr("Wdec1T", [E, 256, 64], bf16)
    dr("id64", [64, 64], bf16)
    dr("sel32", [32, 4], f32)
    dr("benc0", [E, 256], bf16)
    dr("benc1", [E, 256], bf16)
    dr("bdec0", [E, 256], bf16)
    dr("bdec1", [E, 64], bf16)

    out = nc.dram_tensor("out", [64, BC], f32, kind="ExternalOutput")
    if DEBUG:
        dbg_wn = nc.dram_tensor("dbg_wn", [64, BC], f32, kind="ExternalOutput")
        dbg_z0 = nc.dram_tensor("dbg_z0", [256, BC], f32,
                                kind="ExternalOutput")
        dbg_T = nc.dram_tensor("dbg_T", [128, 512], f32, kind="ExternalOutput")
        dbg_ps = nc.dram_tensor("dbg_ps", [256, BC], f32,
                                kind="ExternalOutput")
        dbg_tot = nc.dram_tensor("dbg_tot", [128, 4], f32,
                                 kind="ExternalOutput")

    NB = B // 512   # 8 batch chunks of 512

    with tile.TileContext(nc, num_cores=N_CORES) as tc:
        with tc.tile_pool(name="sb", bufs=1) as sb, \
             tc.tile_pool(name="scr", bufs=3) as scr, \
             tc.tile_pool(name="dram", bufs=1, space="DRAM") as dram:

            # -------- dummy AllGather to absorb collective init / skew ----
            dummy_sb = sb.tile([64, 4], f32)
            nc.vector.memset(dummy_sb[:], 0.0)
            dummy_d = dram.tile([64, 4], f32)
            dummy_g = dram.tile([N_CORES * 64, 4], f32, addr_space="Shared")
            nc.sync.dma_start(dummy_d[:], dummy_sb[:])
            nc.gpsimd.collective_compute(
                "AllGather", OP.bypass,
                replica_groups=[list(range(N_CORES))],
                ins=[dummy_d.opt()], outs=[dummy_g.opt()])

            # ---------------- small param loads ----------------
            def load(name, shape, dt=f32):
                t = sb.tile(list(shape), dt, name=f"L{name}")
                nc.sync.dma_start(t[:], din[name][:])
                return t

            wnt = sb.tile([64, 512], bf16)   # wn^T slice (scaled), bf16

            # ================= MNet (replicated, fp32r) =================
            with tc.tile_pool(name="mbig", bufs=1) as mbig, \
                 tc.tile_pool(name="mps", bufs=4, space="PSUM") as mps, \
                 tc.tile_pool(name="maux", bufs=1, space="PSUM") as maux:
                with tc.tile_pool(name="raw", bufs=1) as raw:
                    w1_raw = raw.tile([128, 256], f32)
                    nc.scalar.dma_start(w1_raw[:], din["mW1T"][:])
                    w1r = sb.tile([128, 256], f32r)
                    nc.vector.tensor_copy(w1r[:], w1_raw[:])
                    m0_raw = raw.tile([128, B], f32)
                    m0r = mbig.tile([128, B], f32r)
                    for j in range(8):
                        if j < 2:  # split first chunks across both engines
                            nc.sync.dma_start(
                                m0_raw[:, 512 * j:512 * j + 256],
                                din["m0T"][:, 512 * j:512 * j + 256])
                            nc.scalar.dma_start(
                                m0_raw[:, 512 * j + 256:512 * (j + 1)],
                                din["m0T"][:, 512 * j + 256:512 * (j + 1)])
                        else:
                            eng = nc.sync if j % 2 == 0 else nc.scalar
                            eng.dma_start(
                                m0_raw[:, 512 * j:512 * (j + 1)],
                                din["m0T"][:, 512 * j:512 * (j + 1)])
                        nc.vector.tensor_copy(
                            m0r[:, 512 * j:512 * (j + 1)],
                            m0_raw[:, 512 * j:512 * (j + 1)])
                    mg1t = load("mg1c", (128, 2))
                    mbe1t = load("mbe1c", (128, 2))
                    ma1t = load("ma1r", (128, 1))
                    mg2t, mbe2t = load("mg2c", (128, 1)), load("mbe2c",
                                                               (128, 1))
                    ma2t = load("ma2r", (128, 1))
                    mg3t, mbe3t = load("mg3c", (64, 1)), load("mbe3c",
                                                              (64, 1))
                    ma3t = load("ma3r", (64, 1))
                    bngt, bnbt = load("bngc", (128, 2)), load("bnbc",
                                                              (128, 2))
                    art = load("ar", (128, 1))
                    x0t = load("x0T", (64, BC), bf16)
                    w2r = []
                    for k in range(2):
                        w2_raw = raw.tile([128, 128], f32, name="w2_raw",
                                          tag="w2_raw")
                        nc.sync.dma_start(w2_raw[:],
                                          din["mW2T"][128 * k:128 * (k + 1), :])
                        w2rk = sb.tile([128, 128], f32r, name=f"w2r_{k}")
                        nc.vector.tensor_copy(w2rk[:], w2_raw[:])
                        w2r.append(w2rk)
                    w3_raw = raw.tile([128, 64], f32)
                    nc.sync.dma_start(w3_raw[:], din["mW3T"][:])
                    w3r = sb.tile([128, 64], f32r)
                    nc.vector.tensor_copy(w3r[:], w3_raw[:])

                # ======== layer 1: 128 -> 256 (single matmul pass) ====
                st1 = sb.tile([128, 4 * NB], f32)  # (s=2, b=NB, c=2)
                y1 = [mbig.tile([128, B], f32, name=f"y1_{c}")
                      for c in range(2)]
                for bc in range(NB):
                    for c in range(2):
                        ps = mps.tile([128, 512], f32, name="mnet_ps")
                        nc.tensor.matmul(ps[:], w1r[:, 128 * c:128 * (c + 1)],
                                         m0r[:, 512 * bc:512 * (bc + 1)],
                                         start=True, stop=True)
                        col = 2 * bc + c
                        nc.vector.tensor_scalar(
                            out=y1[c][:, 512 * bc:512 * (bc + 1)], in0=ps[:],
                            scalar1=1.0, scalar2=0.0, op0=OP.mult,
                            op1=OP.add, accum_out=st1[:, col:col + 1])
                        sq_scr = scr.tile([128, 512], bf16, name="sqscr")
                        nc.scalar.activation(
                            sq_scr[:], ps[:], AF.Square,
                            accum_out=st1[:, 2 * NB + col:
                                          2 * NB + col + 1])
                st1r = sb.tile([128, 4], f32)
                nc.vector.tensor_reduce(
                    st1r[:].rearrange("p (s c) -> p s c", s=2),
                    st1[:].rearrange("p (s b c) -> p s c b", s=2, b=NB, c=2),
                    axis=AX.X, op=OP.add)
                s1, t1 = _bn_apply_params(nc, sb, st1r, 2, B, mg1t[:],
                                          mbe1t[:], "m1")
                h1 = [mbig.tile([128, B], f32r, name=f"h1_{c}")
                      for c in range(2)]
                for bc in range(0, NB, 4):
                    for c in range(2):
                        nc.scalar.activation(
                            h1[c][:, 512 * bc:512 * (bc + 4)],
                            y1[c][:, 512 * bc:512 * (bc + 4)],
                            AF.Prelu, bias=t1[:, c:c + 1],
                            scale=s1[:, c:c + 1], alpha=ma1t[:])

                # ======== layer 2: 256 -> 128 (single pass) ========
                st2 = sb.tile([128, 2 * NB], f32)
                y2 = mbig.tile([128, B], f32)
                for bc in range(NB):
                    ps = mps.tile([128, 512], f32, name="mnet_ps")
                    for k in range(2):
                        nc.tensor.matmul(ps[:], w2r[k][:],
                                         h1[k][:, 512 * bc:512 * (bc + 1)],
                                         start=(k == 0), stop=(k == 1))
                    nc.vector.tensor_scalar(
                        out=y2[:, 512 * bc:512 * (bc + 1)], in0=ps[:],
                        scalar1=1.0, scalar2=0.0, op0=OP.mult,
                        op1=OP.add, accum_out=st2[:, bc:bc + 1])
                    sq_scr = scr.tile([128, 512], bf16, name="sqscr")
                    nc.scalar.activation(sq_scr[:], ps[:], AF.Square,
                                         accum_out=st2[:, NB + bc:NB + bc + 1])
                st2r = sb.tile([128, 2], f32)
                nc.vector.tensor_reduce(
                    st2r[:].rearrange("p (s one) -> p s one", one=1),
                    st2[:].rearrange("p (s b) -> p s b", s=2),
                    axis=AX.X, op=OP.add)
                s2, t2 = _bn_apply_params(nc, sb, st2r, 1, B, mg2t[:],
                                          mbe2t[:], "m2")
                h2 = mbig.tile([128, B], f32r)
                for bc in range(NB):
                    nc.scalar.activation(h2[:, 512 * bc:512 * (bc + 1)],
                                         y2[:, 512 * bc:512 * (bc + 1)],
                                         AF.Prelu, bias=t2[:], scale=s2[:],
                                         alpha=ma2t[:])

                # ======== layer 3: 128 -> 64 (w, single pass) ========
                st3 = sb.tile([64, 2 * NB], f32)
                y3 = mbig.tile([64, B], f32)
                for bc in range(NB):
                    ps = maux.tile([64, 512], f32, name="y3ps", bufs=2)
                    nc.tensor.matmul(ps[:], w3r[:],
                                     h2[:, 512 * bc:512 * (bc + 1)],
                                     start=True, stop=True)
                    nc.vector.tensor_scalar(
                        out=y3[:, 512 * bc:512 * (bc + 1)], in0=ps[:],
                        scalar1=1.0, scalar2=0.0, op0=OP.mult,
                        op1=OP.add, accum_out=st3[:, bc:bc + 1])
                    sq3 = scr.tile([64, 512], bf16, name="sq3scr")
                    nc.scalar.activation(sq3[:], ps[:], AF.Square,
                                         accum_out=st3[:, NB + bc:NB + bc + 1])
                st3r = sb.tile([64, 2], f32)
                nc.vector.tensor_reduce(
                    st3r[:].rearrange("p (s one) -> p s one", one=1),
                    st3[:].rearrange("p (s b) -> p s b", s=2),
                    axis=AX.X, op=OP.add)
                s3, t3 = _bn_apply_params(nc, sb, st3r, 1, B, mg3t[:],
                                          mbe3t[:], "m3")
                w_full = mbig.tile([64, B], f32)
                wrs = sb.tile([64, 1], f32)
                nc.scalar.activation(w_full[:], y3[:], AF.Prelu,
                                     bias=t3[:], scale=s3[:],
                                     alpha=ma3t[:], accum_out=wrs[:])
                ones64 = sb.tile([64, 1], f32)
                nc.vector.memset(ones64[:], 1.0)
                wsum_ps = maux.tile([1, 1], f32, name="wsum_ps", bufs=1)
                nc.tensor.matmul(wsum_ps[:], wrs[:], ones64[:],
                                 start=True, stop=True)
                invw = sb.tile([1, 1], f32)
                nc.vector.reciprocal(invw[:], wsum_ps[:])
                ones1 = sb.tile([1, 64], f32)
                nc.vector.memset(ones1[:], 1.0)
                inv64_ps = maux.tile([64, 1], f32, name="inv64_ps", bufs=1)
                nc.tensor.matmul(inv64_ps[:], ones1[:], invw[:],
                                 start=True, stop=True)
                inv64 = sb.tile([64, 1], f32)
                nc.vector.tensor_copy(inv64[:], inv64_ps[:])
                nc.vector.tensor_scalar(out=wnt[:], in0=w_full[:, 0:512],
                                        scalar1=inv64[:], scalar2=None,
                                        op0=OP.mult)
                if DEBUG:
                    wn_dbg = sb.tile([64, 512], f32)
                    nc.vector.tensor_scalar(out=wn_dbg[:],
                                            in0=w_full[:, 0:512],
                                            scalar1=inv64[:], scalar2=None,
                                            op0=OP.mult)
                    nc.sync.dma_start(dbg_wn[:], wn_dbg[:])

            # ================= expert phase =================
            with tc.tile_pool(name="trep", bufs=1) as trep, \
                 tc.tile_pool(name="wtp", bufs=8) as wtp, \
                 tc.tile_pool(name="zsp", bufs=5) as zsp, \
                 tc.tile_pool(name="pex", bufs=1, space="PSUM") as pex:

                id64t = load("id64", (64, 64), bf16)
                sel32t = load("sel32", (32, 4), f32)
                # T tiles via PE selector matmul + ACT copy (no DMA traffic)
                Tg = trep.tile([128, E * 512], bf16, bufs=1)
                Tg3 = Tg[:].rearrange("p (e b) -> p e b", e=E)
                for e in range(E):
                    tps = pex.tile([128, 512], f32, name="tps", tag="tps",
                                   bufs=2)
                    nc.tensor.matmul(
                        tps[:], id64t[:, e:e + 1].broadcast_to([64, 128]),
                        wnt[:], start=True, stop=True)
                    nc.any.tensor_copy(Tg[:, 512 * e:512 * (e + 1)],
                                       tps[:])

                def stat_allgather(stat, tag):
                    """stat [128, 128] (first 4 cols: sum0,sum1,sq0,sq1) ->
                    summed over cores [128, 4]."""
                    statT = sb.tile([32, 128], f32, name=f"statT_{tag}")
                    for bi in range(4):
                        nc.vector.transpose(
                            statT[0:32, 32 * bi:32 * (bi + 1)],
                            stat[32 * bi:32 * (bi + 1), 0:32])
                    sd = dram.tile([4, 128], f32, name=f"sd_{tag}")
                    gd = dram.tile([N_CORES * 4, 128], f32,
                                   addr_space="Shared", name=f"gd_{tag}")
                    nc.sync.dma_start(sd[:], statT[0:4, :])
                    nc.gpsimd.collective_compute(
                        "AllGather", OP.bypass,
                        replica_groups=[list(range(N_CORES))],
                        ins=[sd.opt()], outs=[gd.opt()])
                    gt32 = sb.tile([32, 128], f32, name=f"gt_{tag}")
                    nc.sync.dma_start(gt32[:], gd[:])
                    totp = pex.tile([128, 4], f32, name=f"totp_{tag}",
                                    tag="totp", bufs=2)
                    nc.tensor.matmul(totp[:], gt32[:], sel32t[:],
                                     start=True, stop=True)
                    tot = sb.tile([128, 4], f32, name=f"tot_{tag}")
                    nc.any.tensor_copy(tot[:], totp[:])
                    return tot

                def expert_layer(zin, kchunks, ochunks, wdram, bias_t, tag,
                                 kpart=128, m=128):
                    """zin: bf16 [kpart, kchunks*512] input tile.
                    Returns list of PSUM tiles [m, 512]."""
                    wfree = wdram.shape[2]
                    psums = [pex.tile([m, 512], f32, name=f"px_{tag}{o}",
                                      tag=f"px{o}", bufs=2)
                             for o in range(ochunks)]
                    zw = kchunks * 512
                    for g in range(E // 4):
                        # zs for 4 experts in one DVE op
                        zs = zsp.tile([kpart, 4 * zw], bf16,
                                      name=f"zs_{tag}", tag="zs")
                        t4 = Tg3[0:kpart, 4 * g:4 * (g + 1), :]
                        zs4 = zs[:].rearrange("p (e k b) -> p e k b", e=4,
                                              k=kchunks)
                        for k in range(kchunks):
                            zin3 = zin[:, 512 * k:512 * (k + 1)] \
                                .unsqueeze(1).broadcast_to([kpart, 4, 512])
                            nc.vector.tensor_tensor(
                                out=zs4[:, :, k, :], in0=zin3, in1=t4,
                                op=OP.mult)
                        # weights: one DMA per 2 experts
                        for h in range(2):
                            wt = wtp.tile([kpart, 2 * kchunks * wfree], bf16,
                                          name=f"wt_{tag}", tag="wt")
                            eng = nc.sync if (h == 0 or tag == "e0") \
                                else nc.scalar
                            e0 = 4 * g + 2 * h
                            if kchunks == 1:
                                eng.dma_start(
                                    wt[:].rearrange("p (e o) -> p e o", e=2),
                                    wdram[e0:e0 + 2].rearrange(
                                        "e p o -> p e o"))
                            else:
                                eng.dma_start(
                                    wt[:].rearrange("p (e k o) -> p e k o",
                                                    e=2, k=kchunks),
                                    wdram[e0:e0 + 2].rearrange(
                                        "e (k p) o -> p e k o", k=kchunks))
                            for j in range(2):
                                ee = 2 * h + j
                                for k in range(kchunks):
                                    for o in range(ochunks):
                                        nc.tensor.matmul(
                                            psums[o][:],
                                            wt[:, kchunks * wfree * j
                                               + wfree * k + 128 * o:
                                               kchunks * wfree * j
                                               + wfree * k + 128 * o + m],
                                            zs[:, zw * ee + 512 * k:
                                               zw * ee + 512 * (k + 1)],
                                            start=(g == 0 and h == 0
                                                   and j == 0 and k == 0),
                                            stop=False)
                    for o in range(ochunks):
                        nc.tensor.matmul(psums[o][:],
                                         bias_t[:, 128 * o:128 * o + m],
                                         wnt[:], start=False, stop=True)
                    return psums

                def bn_prelu(psums, tag):
                    stat = sb.tile([128, 32], f32, name=f"stat_{tag}")
                    nc.vector.memset(stat[:], 0.0)
                    for o in range(2):
                        nc.vector.tensor_reduce(stat[:, o:o + 1], psums[o][:],
                                                axis=AX.X, op=OP.add)
                        sq = scr.tile([128, 512], bf16, name="sqscr")
                        nc.scalar.activation(sq[:], psums[o][:], AF.Square,
                                             accum_out=stat[:, 2 + o:3 + o])
                    tot = stat_allgather(stat, tag)
                    if DEBUG and tag == "e0":
                        nc.sync.dma_start(dbg_tot[:], tot[:])
                    s_ap, t_ap = _bn_apply_params(nc, sb, tot, 2, B, bngt[:],
                                                  bnbt[:], tag,
                                                  prescaled=True)
                    z = sb.tile([128, 1024], bf16, name=f"z_{tag}")
                    for o in range(2):
                        nc.scalar.activation(z[:, 512 * o:512 * (o + 1)],
                                             psums[o][:], AF.Prelu,
                                             bias=t_ap[:, o:o + 1],
                                             scale=s_ap[:, o:o + 1],
                                             alpha=art[:])
                    return z

                benc0t = load("benc0", (E, 256), bf16)
                benc1t = load("benc1", (E, 256), bf16)
                bdec0t = load("bdec0", (E, 256), bf16)
                bdec1t = load("bdec1", (E, 64), bf16)

                ps = expert_layer(x0t[:], 1, 2, Wenc0T, benc0t, "e0",
                                  kpart=64)
                if DEBUG:
                    td = sb.tile([128, 512], f32)
                    nc.vector.tensor_copy(td[:], Tg[:, 0:512])
                    nc.sync.dma_start(dbg_T[:], td[:])
                    for o in range(2):
                        pd = sb.tile([128, 512], f32, name=f"pd{o}")
                        nc.vector.tensor_copy(pd[:], ps[o][:])
                        nc.sync.dma_start(dbg_ps[128 * o:128 * (o + 1), :],
                                          pd[:])
                z = bn_prelu(ps, "e0")
                if DEBUG:
                    for o in range(2):
                        zd = sb.tile([128, 512], f32, name=f"zd{o}")
                        nc.vector.tensor_copy(zd[:],
                                              z[:, 512 * o:512 * (o + 1)])
                        nc.sync.dma_start(dbg_z0[128 * o:128 * (o + 1), :],
                                          zd[:])
                ps = expert_layer(z[:], 2, 2, Wenc1T, benc1t, "e1")
                z = bn_prelu(ps, "e1")
                ps = expert_layer(z[:], 2, 2, Wdec0T, bdec0t, "d0")
                z = bn_prelu(ps, "d0")
                ps = expert_layer(z[:], 2, 1, Wdec1T, bdec1t, "d1", m=64)
                out_sb = sb.tile([64, 512], f32)
                nc.scalar.copy(out_sb[:], ps[0][:])
                nc.sync.dma_start(out[:], out_sb[:])

    nc.compile()
    return nc


def _prep_inputs(inputs):
    """Host-side marshalling: returns per-core in_maps."""
    bf = ml_dtypes.bfloat16
    f = np.float32
    m0 = np.asarray(inputs["m0"], f)
    x0 = np.asarray(inputs["x0"], f)
    m0T_full = np.ascontiguousarray(m0.T)           # [128, 4096]
    x0T_full = np.ascontiguousarray(x0.T)           # [64, 4096]

    def chunk2(v, nch):  # [F] -> [F//nch, nch] chunk-major
        v = np.asarray(v, f)
        p = v.shape[0] // nch
        return np.ascontiguousarray(v.reshape(nch, p).T)

    def rep(v, p):
        return np.full((p, 1), np.asarray(v, f).reshape(-1)[0], f)

    shared = {
        "mW1T": np.ascontiguousarray(np.asarray(inputs["mW1"], f).T),
        "mW2T": np.ascontiguousarray(np.asarray(inputs["mW2"], f).T),
        "mW3T": np.ascontiguousarray(np.asarray(inputs["mW3"], f).T),
        "mg1c": chunk2(inputs["mg1"], 2), "mbe1c": chunk2(inputs["mbe1"], 2),
        "ma1r": rep(inputs["ma1"], 128),
        "mg2c": chunk2(inputs["mg2"], 1), "mbe2c": chunk2(inputs["mbe2"], 1),
        "ma2r": rep(inputs["ma2"], 128),
        "mg3c": chunk2(inputs["mg3"], 1), "mbe3c": chunk2(inputs["mbe3"], 1),
        "ma3r": rep(inputs["ma3"], 64),
        "bngc": chunk2(inputs["bng"], 2), "bnbc": chunk2(inputs["bnb"], 2),
        "ar": rep(inputs["a"], 128),
        "Wenc0T": np.ascontiguousarray(
            np.asarray(inputs["Wenc0"], f).transpose(0, 2, 1)).astype(bf),
        "Wenc1T": np.ascontiguousarray(
            np.asarray(inputs["Wenc1"], f).transpose(0, 2, 1)).astype(bf),
        "Wdec0T": np.ascontiguousarray(
            np.asarray(inputs["Wdec0"], f).transpose(0, 2, 1)).astype(bf),
        "Wdec1T": np.ascontiguousarray(
            np.asarray(inputs["Wdec1"], f).transpose(0, 2, 1)).astype(bf),
        "id64": np.eye(64, dtype=f).astype(bf),
        "sel32": np.tile(np.eye(4, dtype=f) / 4096.0, (8, 1)),
        "benc0": np.asarray(inputs["benc0"], f).astype(bf),
        "benc1": np.asarray(inputs["benc1"], f).astype(bf),
        "bdec0": np.asarray(inputs["bdec0"], f).astype(bf),
        "bdec1": np.asarray(inputs["bdec1"], f).astype(bf),
    }
    in_maps = []
    for i in range(N_CORES):
        r = BC * i
        m0T_rot = np.ascontiguousarray(
            np.concatenate([m0T_full[:, r:], m0T_full[:, :r]], axis=1))
        x0T_sl = np.ascontiguousarray(x0T_full[:, r:r + BC]).astype(bf)
        m = dict(shared)
        m["m0T"] = m0T_rot
        m["x0T"] = x0T_sl
        in_maps.append(m)
    return in_maps


def kernel(**inputs) -> np.ndarray:
    if "nc" not in _cache:
        _cache["nc"] = _build()
    nc = _cache["nc"]
    in_maps = _prep_inputs(inputs)
    res = run_bass_kernel_spmd(nc, in_maps, core_ids=list(range(N_CORES)))
    y = np.empty((B, 64), np.float32)
    for i in range(N_CORES):
        y[BC * i:BC * (i + 1), :] = res.results[i]["out"].T
    return y


# revision 30
# speedup vs baseline: 1.0128x; 1.0128x over previous
"""Trainium2 Bass kernel for nn_GarmentNet_LF (moe_routing).

Strategy (8 NeuronCores, data-parallel over batch):
- MNet (3x Linear+BN+PReLU -> per-sample expert weights) is computed
  REPLICATED on every core over the full batch B=4096 in fp32r (TF32-like)
  so BatchNorm batch statistics need no collectives. Each core's input m0 is
  ROTATED on host so that the core's own 512-sample slice sits at columns
  0:512 (BN stats are permutation-invariant), making slice extraction static.
  MNet linear biases are dropped: training-mode BN directly after a Linear
  absorbs any per-feature constant shift.
- The 4 all-expert dense layers (64 experts) run batch-sharded (512
  samples/core) in bf16. The per-sample expert weighting wn[b,e] is applied
  on the INPUT side: Zs_e = Z * broadcast(wn[:,e]), so the sum over experts
  accumulates for free in PSUM across 64 experts x k-chunks per output bank.
  The expert bias term sum_e wn[b,e] b_e[o] is one extra matmul
  (lhsT=bias[E,O-chunk], rhs=wnT[E,B]) into the same PSUM bank.
- Shared BatchNorm stats for the 3 inner expert layers are combined across
  cores with one small AllGather each (sum & sqsum per feature); a dummy
  AllGather is issued at kernel start to absorb collective init/start skew.
- PReLU+BN apply is a single ScalarE activation (Prelu with per-partition
  scale/bias/alpha) reading PSUM.

Host-side marshalling (numpy): transposes weights to lhsT layout, casts the
expert path to bf16, rotates m0, pre-replicates tiny per-feature BN params.
Device output is feature-major [64, 512] per core; host reassembles.
"""
import numpy as np
import ml_dtypes

import concourse.bacc as bacc
import concourse.mybir as mybir
import concourse.tile as tile
from concourse.bass_utils import run_bass_kernel_spmd

N_CORES = 8
B = 4096
BC = B // N_CORES          # 512 per core
E = 64
EPS = 1e-5

bf16 = mybir.dt.bfloat16
f32 = mybir.dt.float32
f32r = mybir.dt.float32r
AX = mybir.AxisListType
OP = mybir.AluOpType
AF = mybir.ActivationFunctionType

DEBUG = False

_cache = {}


def _bn_apply_params(nc, sb, stat, nfeat_chunks, total_n, g_ap, b_ap, tag,
                     prescaled=False):
    """stat: [P, 2*nfeat_chunks] cols = sums then sqsums (chunk-major).
    If prescaled, stat already holds [mu | e2]."""
    P = stat.shape[0]
    c = nfeat_chunks
    if prescaled:
        me = stat
    else:
        me = sb.tile([P, 2 * c], f32, name=f"me_{tag}")   # [mu | e2]
        nc.vector.tensor_scalar_mul(me[:], stat[:, 0:2 * c], 1.0 / total_n)
    mu = me[:, 0:c]
    mu2 = sb.tile([P, c], f32, name=f"mu2_{tag}")
    nc.vector.tensor_tensor(out=mu2[:], in0=mu, in1=mu, op=OP.mult)
    var = sb.tile([P, c], f32, name=f"var_{tag}")
    nc.vector.tensor_tensor(out=var[:], in0=me[:, c:2 * c], in1=mu2[:],
                            op=OP.subtract)
    nc.vector.tensor_scalar_add(var[:], var[:], EPS)
    std = sb.tile([P, c], f32, name=f"std_{tag}")
    nc.scalar.sqrt(std[:], var[:])
    rstd = sb.tile([P, c], f32, name=f"rstd_{tag}")
    nc.vector.reciprocal(rstd[:], std[:])
    s_ap = sb.tile([P, c], f32, name=f"s_{tag}")
    nc.vector.tensor_tensor(out=s_ap[:], in0=g_ap, in1=rstd[:], op=OP.mult)
    ms = sb.tile([P, c], f32, name=f"ms_{tag}")
    nc.vector.tensor_tensor(out=ms[:], in0=mu, in1=s_ap[:], op=OP.mult)
    t_ap = sb.tile([P, c], f32, name=f"t_{tag}")
    nc.vector.tensor_tensor(out=t_ap[:], in0=b_ap, in1=ms[:], op=OP.subtract)
    return s_ap, t_ap



def _build():
    nc = bacc.Bacc("TRN2", target_bir_lowering=False, debug=False,
                   enable_asserts=False, num_devices=N_CORES)

    # ---------------- DRAM parameters ----------------
    din = {}

    def dr(name, shape, dt):
        din[name] = nc.dram_tensor(name, shape, dt, kind="ExternalInput")
        return din[name]

    dr("m0T", [128, B], f32)            # rotated per core
    dr("x0T", [64, BC], bf16)           # core's slice, feature-major
    dr("mW1T", [128, 256], f32)
    dr("mW2T", [256, 128], f32)
    dr("mW3T", [128, 64], f32)
    dr("mg1c", [128, 2], f32)
    dr("mbe1c", [128, 2], f32)
    dr("ma1r", [128, 1], f32)
    dr("mg2c", [128, 1], f32)
    dr("mbe2c", [128, 1], f32)
    dr("ma2r", [128, 1], f32)
    dr("mg3c", [64, 1], f32)
    dr("mbe3c", [64, 1], f32)
    dr("ma3r", [64, 1], f32)
    dr("bngc", [128, 2], f32)
    dr("bnbc", [128, 2], f32)
    dr("ar", [128, 1], f32)
    Wenc0T = dr("Wenc0T", [E, 64, 256], bf16)
    Wenc1T = dr("Wenc1T", [E, 256, 256], bf16)
    Wdec0T = dr("Wdec0T", [E, 256, 256], bf16)
    Wdec1T = dr("Wdec1T", [E, 256, 64], bf16)
    dr("id64", [64, 64], bf16)
    dr("sel32", [32, 4], f32)
    dr("benc0", [E, 256], bf16)
    dr("benc1", [E, 256], bf16)
    dr("bdec0", [E, 256], bf16)
    dr("bdec1", [E, 64], bf16)

    out = nc.dram_tensor("out", [64, BC], f32, kind="ExternalOutput")
    if DEBUG:
        dbg_wn = nc.dram_tensor("dbg_wn", [64, BC], f32, kind="ExternalOutput")
        dbg_z0 = nc.dram_tensor("dbg_z0", [256, BC], f32,
                                kind="ExternalOutput")
        dbg_T = nc.dram_tensor("dbg_T", [128, 512], f32, kind="ExternalOutput")
        dbg_ps = nc.dram_tensor("dbg_ps", [256, BC], f32,
                                kind="ExternalOutput")
        dbg_tot = nc.dram_tensor("dbg_tot", [128, 4], f32,
                                 kind="ExternalOutput")

    NB = B // 512   # 8 batch chunks of 512

    with tile.TileContext(nc, num_cores=N_CORES) as tc:
        with tc.tile_pool(name="sb", bufs=1) as sb, \
             tc.tile_pool(name="scr", bufs=3) as scr, \
             tc.tile_pool(name="dram", bufs=1, space="DRAM") as dram:

            # -------- dummy AllGather to absorb collective init / skew ----
            dummy_sb = sb.tile([64, 4], f32)
            nc.vector.memset(dummy_sb[:], 0.0)
            dummy_d = dram.tile([64, 4], f32)
            dummy_g = dram.tile([N_CORES * 64, 4], f32, addr_space="Shared")
            nc.sync.dma_start(dummy_d[:], dummy_sb[:])
            nc.gpsimd.collective_compute(
                "AllGather", OP.bypass,
                replica_groups=[list(range(N_CORES))],
                ins=[dummy_d.opt()], outs=[dummy_g.opt()])

            # ---------------- small param loads ----------------
            def load(name, shape, dt=f32):
                t = sb.tile(list(shape), dt, name=f"L{name}")
                nc.sync.dma_start(t[:], din[name][:])
                return t

            wnt = sb.tile([64, 512], bf16)   # wn^T slice (scaled), bf16

            # ================= MNet (replicated, fp32r) =================
            with tc.tile_pool(name="mbig", bufs=1) as mbig, \
                 tc.tile_pool(name="mps", bufs=4, space="PSUM") as mps, \
                 tc.tile_pool(name="maux", bufs=1, space="PSUM") as maux:
                with tc.tile_pool(name="raw", bufs=1) as raw:
                    w1_raw = raw.tile([128, 256], f32)
                    nc.scalar.dma_start(w1_raw[:], din["mW1T"][:])
                    w1r = sb.tile([128, 256], f32r)
                    nc.vector.tensor_copy(w1r[:], w1_raw[:])
                    m0_raw = raw.tile([128, B], f32)
                    m0r = mbig.tile([128, B], f32r)
                    for j in range(8):
                        if j < 2:  # split first chunks across both engines
                            nc.sync.dma_start(
                                m0_raw[:, 512 * j:512 * j + 256],
                                din["m0T"][:, 512 * j:512 * j + 256])
                            nc.scalar.dma_start(
                                m0_raw[:, 512 * j + 256:512 * (j + 1)],
                                din["m0T"][:, 512 * j + 256:512 * (j + 1)])
                        else:
                            eng = nc.sync if j % 2 == 0 else nc.scalar
                            eng.dma_start(
                                m0_raw[:, 512 * j:512 * (j + 1)],
                                din["m0T"][:, 512 * j:512 * (j + 1)])
                        nc.vector.tensor_copy(
                            m0r[:, 512 * j:512 * (j + 1)],
                            m0_raw[:, 512 * j:512 * (j + 1)])
                    mg1t = load("mg1c", (128, 2))
                    mbe1t = load("mbe1c", (128, 2))
                    ma1t = load("ma1r", (128, 1))
                    mg2t, mbe2t = load("mg2c", (128, 1)), load("mbe2c",
                                                               (128, 1))
                    ma2t = load("ma2r", (128, 1))
                    mg3t, mbe3t = load("mg3c", (64, 1)), load("mbe3c",
                                                              (64, 1))
                    ma3t = load("ma3r", (64, 1))
                    bngt, bnbt = load("bngc", (128, 2)), load("bnbc",
                                                              (128, 2))
                    art = load("ar", (128, 1))
                    x0t = load("x0T", (64, BC), bf16)
                    id64t = load("id64", (64, 64), bf16)
                    sel32t = load("sel32", (32, 4), f32)
                    benc0t = load("benc0", (E, 256), bf16)
                    benc1t = load("benc1", (E, 256), bf16)
                    bdec0t = load("bdec0", (E, 256), bf16)
                    bdec1t = load("bdec1", (E, 64), bf16)
                    w2r = []
                    for k in range(2):
                        w2_raw = raw.tile([128, 128], f32, name="w2_raw",
                                          tag="w2_raw")
                        nc.sync.dma_start(w2_raw[:],
                                          din["mW2T"][128 * k:128 * (k + 1), :])
                        w2rk = sb.tile([128, 128], f32r, name=f"w2r_{k}")
                        nc.vector.tensor_copy(w2rk[:], w2_raw[:])
                        w2r.append(w2rk)
                    w3_raw = raw.tile([128, 64], f32)
                    nc.sync.dma_start(w3_raw[:], din["mW3T"][:])
                    w3r = sb.tile([128, 64], f32r)
                    nc.vector.tensor_copy(w3r[:], w3_raw[:])

                # ======== layer 1: 128 -> 256 (single matmul pass) ====
                st1 = sb.tile([128, 4 * NB], f32)  # (s=2, b=NB, c=2)
                y1 = [mbig.tile([128, B], f32, name=f"y1_{c}")
                      for c in range(2)]
                for bc in range(NB):
                    for c in range(2):
                        ps = mps.tile([128, 512], f32, name="mnet_ps")
                        nc.tensor.matmul(ps[:], w1r[:, 128 * c:128 * (c + 1)],
                                         m0r[:, 512 * bc:512 * (bc + 1)],
                                         start=True, stop=True)
                        col = 2 * bc + c
                        nc.vector.tensor_scalar(
                            out=y1[c][:, 512 * bc:512 * (bc + 1)], in0=ps[:],
                            scalar1=1.0, scalar2=0.0, op0=OP.mult,
                            op1=OP.add, accum_out=st1[:, col:col + 1])
                        sq_scr = scr.tile([128, 512], bf16, name="sqscr")
                        nc.scalar.activation(
                            sq_scr[:], ps[:], AF.Square,
                            accum_out=st1[:, 2 * NB + col:
                                          2 * NB + col + 1])
                st1r = sb.tile([128, 4], f32)
                nc.vector.tensor_reduce(
                    st1r[:].rearrange("p (s c) -> p s c", s=2),
                    st1[:].rearrange("p (s b c) -> p s c b", s=2, b=NB, c=2),
                    axis=AX.X, op=OP.add)
                s1, t1 = _bn_apply_params(nc, sb, st1r, 2, B, mg1t[:],
                                          mbe1t[:], "m1")
                h1 = [mbig.tile([128, B], f32r, name=f"h1_{c}")
                      for c in range(2)]
                for bc in range(0, NB, 4):
                    for c in range(2):
                        nc.scalar.activation(
                            h1[c][:, 512 * bc:512 * (bc + 4)],
                            y1[c][:, 512 * bc:512 * (bc + 4)],
                            AF.Prelu, bias=t1[:, c:c + 1],
                            scale=s1[:, c:c + 1], alpha=ma1t[:])

                # ======== layer 2: 256 -> 128 (single pass) ========
                st2 = sb.tile([128, 2 * NB], f32)
                y2 = mbig.tile([128, B], f32)
                for bc in range(NB):
                    ps = mps.tile([128, 512], f32, name="mnet_ps")
                    for k in range(2):
                        nc.tensor.matmul(ps[:], w2r[k][:],
                                         h1[k][:, 512 * bc:512 * (bc + 1)],
                                         start=(k == 0), stop=(k == 1))
                    nc.vector.tensor_scalar(
                        out=y2[:, 512 * bc:512 * (bc + 1)], in0=ps[:],
                        scalar1=1.0, scalar2=0.0, op0=OP.mult,
                        op1=OP.add, accum_out=st2[:, bc:bc + 1])
                    sq_scr = scr.tile([128, 512], bf16, name="sqscr")
                    nc.scalar.activation(sq_scr[:], ps[:], AF.Square,
                                         accum_out=st2[:, NB + bc:NB + bc + 1])
                st2r = sb.tile([128, 2], f32)
                nc.vector.tensor_reduce(
                    st2r[:].rearrange("p (s one) -> p s one", one=1),
                    st2[:].rearrange("p (s b) -> p s b", s=2),
                    axis=AX.X, op=OP.add)
                s2, t2 = _bn_apply_params(nc, sb, st2r, 1, B, mg2t[:],
                                          mbe2t[:], "m2")
                h2 = mbig.tile([128, B], f32r)
                for bc in range(NB):
                    nc.scalar.activation(h2[:, 512 * bc:512 * (bc + 1)],
                                         y2[:, 512 * bc:512 * (bc + 1)],
                                         AF.Prelu, bias=t2[:], scale=s2[:],
                                         alpha=ma2t[:])

                # ======== layer 3: 128 -> 64 (w, single pass) ========
                st3 = sb.tile([64, 2 * NB], f32)
                y3 = mbig.tile([64, B], f32)
                for bc in range(NB):
                    ps = maux.tile([64, 512], f32, name="y3ps", bufs=2)
                    nc.tensor.matmul(ps[:], w3r[:],
                                     h2[:, 512 * bc:512 * (bc + 1)],
                                     start=True, stop=True)
                    nc.vector.tensor_scalar(
                        out=y3[:, 512 * bc:512 * (bc + 1)], in0=ps[:],
                        scalar1=1.0, scalar2=0.0, op0=OP.mult,
                        op1=OP.add, accum_out=st3[:, bc:bc + 1])
                    sq3 = scr.tile([64, 512], bf16, name="sq3scr")
                    nc.scalar.activation(sq3[:], ps[:], AF.Square,
                                         accum_out=st3[:, NB + bc:NB + bc + 1])
                st3r = sb.tile([64, 2], f32)
                nc.vector.tensor_reduce(
                    st3r[:].rearrange("p (s one) -> p s one", one=1),
                    st3[:].rearrange("p (s b) -> p s b", s=2),
                    axis=AX.X, op=OP.add)
                s3, t3 = _bn_apply_params(nc, sb, st3r, 1, B, mg3t[:],
                                          mbe3t[:], "m3")
                w_full = mbig.tile([64, B], f32)
                wrs = sb.tile([64, 1], f32)
                nc.scalar.activation(w_full[:], y3[:], AF.Prelu,
                                     bias=t3[:], scale=s3[:],
                                     alpha=ma3t[:], accum_out=wrs[:])
                ones64 = sb.tile([64, 1], f32)
                nc.vector.memset(ones64[:], 1.0)
                wsum_ps = maux.tile([1, 1], f32, name="wsum_ps", bufs=1)
                nc.tensor.matmul(wsum_ps[:], wrs[:], ones64[:],
                                 start=True, stop=True)
                invw = sb.tile([1, 1], f32)
                nc.vector.reciprocal(invw[:], wsum_ps[:])
                ones1 = sb.tile([1, 64], f32)
                nc.vector.memset(ones1[:], 1.0)
                inv64_ps = maux.tile([64, 1], f32, name="inv64_ps", bufs=1)
                nc.tensor.matmul(inv64_ps[:], ones1[:], invw[:],
                                 start=True, stop=True)
                inv64 = sb.tile([64, 1], f32)
                nc.vector.tensor_copy(inv64[:], inv64_ps[:])
                nc.vector.tensor_scalar(out=wnt[:], in0=w_full[:, 0:512],
                                        scalar1=inv64[:], scalar2=None,
                                        op0=OP.mult)
                if DEBUG:
                    wn_dbg = sb.tile([64, 512], f32)
                    nc.vector.tensor_scalar(out=wn_dbg[:],
                                            in0=w_full[:, 0:512],
                                            scalar1=inv64[:], scalar2=None,
                                            op0=OP.mult)
                    nc.sync.dma_start(dbg_wn[:], wn_dbg[:])

            # ================= expert phase =================
            with tc.tile_pool(name="trep", bufs=1) as trep, \
                 tc.tile_pool(name="wtp", bufs=8) as wtp, \
                 tc.tile_pool(name="zsp", bufs=5) as zsp, \
                 tc.tile_pool(name="pex", bufs=1, space="PSUM") as pex:

                # T tiles via PE selector matmul + ACT copy (no DMA traffic)
                Tg = trep.tile([128, E * 512], bf16, bufs=1)
                Tg3 = Tg[:].rearrange("p (e b) -> p e b", e=E)
                for e in range(E):
                    tps = pex.tile([128, 512], f32, name="tps", tag="tps",
                                   bufs=2)
                    nc.tensor.matmul(
                        tps[:], id64t[:, e:e + 1].broadcast_to([64, 128]),
                        wnt[:], start=True, stop=True)
                    nc.any.tensor_copy(Tg[:, 512 * e:512 * (e + 1)],
                                       tps[:])

                def stat_allgather(stat, tag):
                    """stat [128, 128] (first 4 cols: sum0,sum1,sq0,sq1) ->
                    summed over cores [128, 4]."""
                    statT = sb.tile([32, 128], f32, name=f"statT_{tag}")
                    for bi in range(4):
                        nc.vector.transpose(
                            statT[0:32, 32 * bi:32 * (bi + 1)],
                            stat[32 * bi:32 * (bi + 1), 0:32])
                    sd = dram.tile([4, 128], f32, name=f"sd_{tag}")
                    gd = dram.tile([N_CORES * 4, 128], f32,
                                   addr_space="Shared", name=f"gd_{tag}")
                    nc.sync.dma_start(sd[:], statT[0:4, :])
                    nc.gpsimd.collective_compute(
                        "AllGather", OP.bypass,
                        replica_groups=[list(range(N_CORES))],
                        ins=[sd.opt()], outs=[gd.opt()])
                    gt32 = sb.tile([32, 128], f32, name=f"gt_{tag}")
                    nc.sync.dma_start(gt32[:], gd[:])
                    totp = pex.tile([128, 4], f32, name=f"totp_{tag}",
                                    tag="totp", bufs=2)
                    nc.tensor.matmul(totp[:], gt32[:], sel32t[:],
                                     start=True, stop=True)
                    tot = sb.tile([128, 4], f32, name=f"tot_{tag}")
                    nc.any.tensor_copy(tot[:], totp[:])
                    return tot

                def expert_layer(zin, kchunks, ochunks, wdram, bias_t, tag,
                                 kpart=128, m=128):
                    """zin: bf16 [kpart, kchunks*512] input tile.
                    Returns list of PSUM tiles [m, 512]."""
                    wfree = wdram.shape[2]
                    psums = [pex.tile([m, 512], f32, name=f"px_{tag}{o}",
                                      tag=f"px{o}", bufs=2)
                             for o in range(ochunks)]
                    zw = kchunks * 512
                    for g in range(E // 4):
                        # zs for 4 experts in one DVE op
                        zs = zsp.tile([kpart, 4 * zw], bf16,
                                      name=f"zs_{tag}", tag="zs")
                        t4 = Tg3[0:kpart, 4 * g:4 * (g + 1), :]
                        zs4 = zs[:].rearrange("p (e k b) -> p e k b", e=4,
                                              k=kchunks)
                        for k in range(kchunks):
                            zin3 = zin[:, 512 * k:512 * (k + 1)] \
                                .unsqueeze(1).broadcast_to([kpart, 4, 512])
                            nc.vector.tensor_tensor(
                                out=zs4[:, :, k, :], in0=zin3, in1=t4,
                                op=OP.mult)
                        # weights: one DMA per 2 experts
                        for h in range(2):
                            wt = wtp.tile([kpart, 2 * kchunks * wfree], bf16,
                                          name=f"wt_{tag}", tag="wt")
                            eng = nc.sync if (h == 0 or tag == "e0") \
                                else nc.scalar
                            e0 = 4 * g + 2 * h
                            if kchunks == 1:
                                eng.dma_start(
                                    wt[:].rearrange("p (e o) -> p e o", e=2),
                                    wdram[e0:e0 + 2].rearrange(
                                        "e p o -> p e o"))
                            else:
                                eng.dma_start(
                                    wt[:].rearrange("p (e k o) -> p e k o",
                                                    e=2, k=kchunks),
                                    wdram[e0:e0 + 2].rearrange(
                                        "e (k p) o -> p e k o", k=kchunks))
                            for j in range(2):
                                ee = 2 * h + j
                                for k in range(kchunks):
                                    for o in range(ochunks):
                                        nc.tensor.matmul(
                                            psums[o][:],
                                            wt[:, kchunks * wfree * j
                                               + wfree * k + 128 * o:
                                               kchunks * wfree * j
                                               + wfree * k + 128 * o + m],
                                            zs[:, zw * ee + 512 * k:
                                               zw * ee + 512 * (k + 1)],
                                            start=(g == 0 and h == 0
                                                   and j == 0 and k == 0),
                                            stop=False)
                    for o in range(ochunks):
                        nc.tensor.matmul(psums[o][:],
                                         bias_t[:, 128 * o:128 * o + m],
                                         wnt[:], start=False, stop=True)
                    return psums

                def bn_prelu(psums, tag):
                    stat = sb.tile([128, 32], f32, name=f"stat_{tag}")
                    nc.vector.memset(stat[:], 0.0)
                    for o in range(2):
                        nc.vector.tensor_reduce(stat[:, o:o + 1], psums[o][:],
                                                axis=AX.X, op=OP.add)
                        sq = scr.tile([128, 512], bf16, name="sqscr")
                        nc.scalar.activation(sq[:], psums[o][:], AF.Square,
                                             accum_out=stat[:, 2 + o:3 + o])
                    tot = stat_allgather(stat, tag)
                    if DEBUG and tag == "e0":
                        nc.sync.dma_start(dbg_tot[:], tot[:])
                    s_ap, t_ap = _bn_apply_params(nc, sb, tot, 2, B, bngt[:],
                                                  bnbt[:], tag,
                                                  prescaled=True)
                    z = sb.tile([128, 1024], bf16, name=f"z_{tag}")
                    for o in range(2):
                        nc.scalar.activation(z[:, 512 * o:512 * (o + 1)],
                                             psums[o][:], AF.Prelu,
                                             bias=t_ap[:, o:o + 1],
                                             scale=s_ap[:, o:o + 1],
                                             alpha=art[:])
                    return z

                ps = expert_layer(x0t[:], 1, 2, Wenc0T, benc0t, "e0",
                                  kpart=64)
                if DEBUG:
                    td = sb.tile([128, 512], f32)
                    nc.vector.tensor_copy(td[:], Tg[:, 0:512])
                    nc.sync.dma_start(dbg_T[:], td[:])
                    for o in range(2):
                        pd = sb.tile([128, 512], f32, name=f"pd{o}")
                        nc.vector.tensor_copy(pd[:], ps[o][:])
                        nc.sync.dma_start(dbg_ps[128 * o:128 * (o + 1), :],
                                          pd[:])
                z = bn_prelu(ps, "e0")
                if DEBUG:
                    for o in range(2):
                        zd = sb.tile([128, 512], f32, name=f"zd{o}")
                        nc.vector.tensor_copy(zd[:],
                                              z[:, 512 * o:512 * (o + 1)])
                        nc.sync.dma_start(dbg_z0[128 * o:128 * (o + 1), :],
                                          zd[:])
                ps = expert_layer(z[:], 2, 2, Wenc1T, benc1t, "e1")
                z = bn_prelu(ps, "e1")
                ps = expert_layer(z[:], 2, 2, Wdec0T, bdec0t, "d0")
                z = bn_prelu(ps, "d0")
                ps = expert_layer(z[:], 2, 1, Wdec1T, bdec1t, "d1", m=64)
                out_sb = sb.tile([64, 512], f32)
                nc.scalar.copy(out_sb[:], ps[0][:])
                nc.sync.dma_start(out[:], out_sb[:])

    nc.compile()
    return nc


def _prep_inputs(inputs):
    """Host-side marshalling: returns per-core in_maps."""
    bf = ml_dtypes.bfloat16
    f = np.float32
    m0 = np.asarray(inputs["m0"], f)
    x0 = np.asarray(inputs["x0"], f)
    m0T_full = np.ascontiguousarray(m0.T)           # [128, 4096]
    x0T_full = np.ascontiguousarray(x0.T)           # [64, 4096]

    def chunk2(v, nch):  # [F] -> [F//nch, nch] chunk-major
        v = np.asarray(v, f)
        p = v.shape[0] // nch
        return np.ascontiguousarray(v.reshape(nch, p).T)

    def rep(v, p):
        return np.full((p, 1), np.asarray(v, f).reshape(-1)[0], f)

    shared = {
        "mW1T": np.ascontiguousarray(np.asarray(inputs["mW1"], f).T),
        "mW2T": np.ascontiguousarray(np.asarray(inputs["mW2"], f).T),
        "mW3T": np.ascontiguousarray(np.asarray(inputs["mW3"], f).T),
        "mg1c": chunk2(inputs["mg1"], 2), "mbe1c": chunk2(inputs["mbe1"], 2),
        "ma1r": rep(inputs["ma1"], 128),
        "mg2c": chunk2(inputs["mg2"], 1), "mbe2c": chunk2(inputs["mbe2"], 1),
        "ma2r": rep(inputs["ma2"], 128),
        "mg3c": chunk2(inputs["mg3"], 1), "mbe3c": chunk2(inputs["mbe3"], 1),
        "ma3r": rep(inputs["ma3"], 64),
        "bngc": chunk2(inputs["bng"], 2), "bnbc": chunk2(inputs["bnb"], 2),
        "ar": rep(inputs["a"], 128),
        "Wenc0T": np.ascontiguousarray(
            np.asarray(inputs["Wenc0"], f).transpose(0, 2, 1)).astype(bf),
        "Wenc1T": np.ascontiguousarray(
            np.asarray(inputs["Wenc1"], f).transpose(0, 2, 1)).astype(bf),
        "Wdec0T": np.ascontiguousarray(
            np.asarray(inputs["Wdec0"], f).transpose(0, 2, 1)).astype(bf),
        "Wdec1T": np.ascontiguousarray(
            np.asarray(inputs["Wdec1"], f).transpose(0, 2, 1)).astype(bf),
        "id64": np.eye(64, dtype=f).astype(bf),
        "sel32": np.tile(np.eye(4, dtype=f) / 4096.0, (8, 1)),
        "benc0": np.asarray(inputs["benc0"], f).astype(bf),
        "benc1": np.asarray(inputs["benc1"], f).astype(bf),
        "bdec0": np.asarray(inputs["bdec0"], f).astype(bf),
        "bdec1": np.asarray(inputs["bdec1"], f).astype(bf),
    }
    in_maps = []
    for i in range(N_CORES):
        r = BC * i
        m0T_rot = np.ascontiguousarray(
            np.concatenate([m0T_full[:, r:], m0T_full[:, :r]], axis=1))
        x0T_sl = np.ascontiguousarray(x0T_full[:, r:r + BC]).astype(bf)
        m = dict(shared)
        m["m0T"] = m0T_rot
        m["x0T"] = x0T_sl
        in_maps.append(m)
    return in_maps


def kernel(**inputs) -> np.ndarray:
    if "nc" not in _cache:
        _cache["nc"] = _build()
    nc = _cache["nc"]
    in_maps = _prep_inputs(inputs)
    res = run_bass_kernel_spmd(nc, in_maps, core_ids=list(range(N_CORES)))
    y = np.empty((B, 64), np.float32)
    for i in range(N_CORES):
        y[BC * i:BC * (i + 1), :] = res.results[i]["out"].T
    return y


# revision 31
# speedup vs baseline: 1.0262x; 1.0133x over previous
"""Trainium2 Bass kernel for nn_GarmentNet_LF (moe_routing).

Strategy (8 NeuronCores, data-parallel over batch):
- MNet (3x Linear+BN+PReLU -> per-sample expert weights) is computed
  REPLICATED on every core over the full batch B=4096 in fp32r (TF32-like)
  so BatchNorm batch statistics need no collectives. Each core's input m0 is
  ROTATED on host so that the core's own 512-sample slice sits at columns
  0:512 (BN stats are permutation-invariant), making slice extraction static.
  MNet linear biases are dropped: training-mode BN directly after a Linear
  absorbs any per-feature constant shift.
- The 4 all-expert dense layers (64 experts) run batch-sharded (512
  samples/core) in bf16. The per-sample expert weighting wn[b,e] is applied
  on the INPUT side: Zs_e = Z * broadcast(wn[:,e]), so the sum over experts
  accumulates for free in PSUM across 64 experts x k-chunks per output bank.
  The expert bias term sum_e wn[b,e] b_e[o] is one extra matmul
  (lhsT=bias[E,O-chunk], rhs=wnT[E,B]) into the same PSUM bank.
- Shared BatchNorm stats for the 3 inner expert layers are combined across
  cores with one small AllGather each (sum & sqsum per feature); a dummy
  AllGather is issued at kernel start to absorb collective init/start skew.
- PReLU+BN apply is a single ScalarE activation (Prelu with per-partition
  scale/bias/alpha) reading PSUM.

Host-side marshalling (numpy): transposes weights to lhsT layout, casts the
expert path to bf16, rotates m0, pre-replicates tiny per-feature BN params.
Device output is feature-major [64, 512] per core; host reassembles.
"""
import numpy as np
import ml_dtypes

import concourse.bacc as bacc
import concourse.mybir as mybir
import concourse.tile as tile
from concourse.bass_utils import run_bass_kernel_spmd

N_CORES = 8
B = 4096
BC = B // N_CORES          # 512 per core
E = 64
EPS = 1e-5

bf16 = mybir.dt.bfloat16
f32 = mybir.dt.float32
f32r = mybir.dt.float32r
AX = mybir.AxisListType
OP = mybir.AluOpType
AF = mybir.ActivationFunctionType

DEBUG = False

_cache = {}


def _bn_apply_params(nc, sb, stat, nfeat_chunks, total_n, g_ap, b_ap, tag,
                     prescaled=False):
    """stat: [P, 2*nfeat_chunks] cols = sums then sqsums (chunk-major).
    If prescaled, stat already holds [mu | e2]."""
    P = stat.shape[0]
    c = nfeat_chunks
    if prescaled:
        me = stat
    else:
        me = sb.tile([P, 2 * c], f32, name=f"me_{tag}")   # [mu | e2]
        nc.vector.tensor_scalar_mul(me[:], stat[:, 0:2 * c], 1.0 / total_n)
    mu = me[:, 0:c]
    mu2 = sb.tile([P, c], f32, name=f"mu2_{tag}")
    nc.vector.tensor_tensor(out=mu2[:], in0=mu, in1=mu, op=OP.mult)
    var = sb.tile([P, c], f32, name=f"var_{tag}")
    nc.vector.tensor_tensor(out=var[:], in0=me[:, c:2 * c], in1=mu2[:],
                            op=OP.subtract)
    nc.vector.tensor_scalar_add(var[:], var[:], EPS)
    std = sb.tile([P, c], f32, name=f"std_{tag}")
    nc.scalar.sqrt(std[:], var[:])
    rstd = sb.tile([P, c], f32, name=f"rstd_{tag}")
    nc.vector.reciprocal(rstd[:], std[:])
    s_ap = sb.tile([P, c], f32, name=f"s_{tag}")
    nc.vector.tensor_tensor(out=s_ap[:], in0=g_ap, in1=rstd[:], op=OP.mult)
    ms = sb.tile([P, c], f32, name=f"ms_{tag}")
    nc.vector.tensor_tensor(out=ms[:], in0=mu, in1=s_ap[:], op=OP.mult)
    t_ap = sb.tile([P, c], f32, name=f"t_{tag}")
    nc.vector.tensor_tensor(out=t_ap[:], in0=b_ap, in1=ms[:], op=OP.subtract)
    return s_ap, t_ap



def _build():
    nc = bacc.Bacc("TRN2", target_bir_lowering=False, debug=False,
                   enable_asserts=False, num_devices=N_CORES)

    # ---------------- DRAM parameters ----------------
    din = {}

    def dr(name, shape, dt):
        din[name] = nc.dram_tensor(name, shape, dt, kind="ExternalInput")
        return din[name]

    dr("m0T", [128, B], f32)            # rotated per core
    dr("x0T", [64, BC], bf16)           # core's slice, feature-major
    dr("mW1T", [128, 256], f32)
    dr("mW2T", [256, 128], f32)
    dr("mW3T", [128, 64], f32)
    dr("mg1c", [128, 2], f32)
    dr("mbe1c", [128, 2], f32)
    dr("ma1r", [128, 1], f32)
    dr("mg2c", [128, 1], f32)
    dr("mbe2c", [128, 1], f32)
    dr("ma2r", [128, 1], f32)
    dr("mg3c", [64, 1], f32)
    dr("mbe3c", [64, 1], f32)
    dr("ma3r", [64, 1], f32)
    dr("bngc", [128, 2], f32)
    dr("bnbc", [128, 2], f32)
    dr("ar", [128, 1], f32)
    Wenc0T = dr("Wenc0T", [E, 64, 256], bf16)
    Wenc1T = dr("Wenc1T", [E, 256, 256], bf16)
    Wdec0T = dr("Wdec0T", [E, 256, 256], bf16)
    Wdec1T = dr("Wdec1T", [E, 256, 64], bf16)
    dr("id64", [64, 64], bf16)
    dr("sel32", [32, 4], f32)
    dr("benc0", [E, 256], bf16)
    dr("benc1", [E, 256], bf16)
    dr("bdec0", [E, 256], bf16)
    dr("bdec1", [E, 64], bf16)

    out = nc.dram_tensor("out", [64, BC], f32, kind="ExternalOutput")
    if DEBUG:
        dbg_wn = nc.dram_tensor("dbg_wn", [64, BC], f32, kind="ExternalOutput")
        dbg_z0 = nc.dram_tensor("dbg_z0", [256, BC], f32,
                                kind="ExternalOutput")
        dbg_T = nc.dram_tensor("dbg_T", [128, 512], f32, kind="ExternalOutput")
        dbg_ps = nc.dram_tensor("dbg_ps", [256, BC], f32,
                                kind="ExternalOutput")
        dbg_tot = nc.dram_tensor("dbg_tot", [128, 4], f32,
                                 kind="ExternalOutput")

    NB = B // 512   # 8 batch chunks of 512

    with tile.TileContext(nc, num_cores=N_CORES) as tc:
        with tc.tile_pool(name="sb", bufs=1) as sb, \
             tc.tile_pool(name="scr", bufs=3) as scr, \
             tc.tile_pool(name="dram", bufs=1, space="DRAM") as dram:

            # -------- dummy AllGather to absorb collective init / skew ----
            dummy_sb = sb.tile([64, 4], f32)
            nc.vector.memset(dummy_sb[:], 0.0)
            dummy_d = dram.tile([64, 4], f32)
            dummy_g = dram.tile([N_CORES * 64, 4], f32, addr_space="Shared")
            nc.sync.dma_start(dummy_d[:], dummy_sb[:])
            nc.gpsimd.collective_compute(
                "AllGather", OP.bypass,
                replica_groups=[list(range(N_CORES))],
                ins=[dummy_d.opt()], outs=[dummy_g.opt()])
            # second dummy with the exact real stat-AG shape warms the
            # steady-state ncfw path before the first real sync
            dummy2_sb = sb.tile([4, 128], f32)
            nc.vector.memset(dummy2_sb[:], 0.0)
            dummy2_d = dram.tile([4, 128], f32)
            dummy2_g = dram.tile([N_CORES * 4, 128], f32, addr_space="Shared")
            nc.sync.dma_start(dummy2_d[:], dummy2_sb[:])
            nc.gpsimd.collective_compute(
                "AllGather", OP.bypass,
                replica_groups=[list(range(N_CORES))],
                ins=[dummy2_d.opt()], outs=[dummy2_g.opt()])

            # ---------------- small param loads ----------------
            def load(name, shape, dt=f32):
                t = sb.tile(list(shape), dt, name=f"L{name}")
                nc.sync.dma_start(t[:], din[name][:])
                return t

            wnt = sb.tile([64, 512], bf16)   # wn^T slice (scaled), bf16

            # ================= MNet (replicated, fp32r) =================
            with tc.tile_pool(name="mbig", bufs=1) as mbig, \
                 tc.tile_pool(name="mps", bufs=4, space="PSUM") as mps, \
                 tc.tile_pool(name="maux", bufs=1, space="PSUM") as maux:
                with tc.tile_pool(name="raw", bufs=1) as raw:
                    w1_raw = raw.tile([128, 256], f32)
                    nc.scalar.dma_start(w1_raw[:], din["mW1T"][:])
                    w1r = sb.tile([128, 256], f32r)
                    nc.vector.tensor_copy(w1r[:], w1_raw[:])
                    m0_raw = raw.tile([128, B], f32)
                    m0r = mbig.tile([128, B], f32r)
                    for j in range(8):
                        if j < 2:  # split first chunks across both engines
                            nc.sync.dma_start(
                                m0_raw[:, 512 * j:512 * j + 256],
                                din["m0T"][:, 512 * j:512 * j + 256])
                            nc.scalar.dma_start(
                                m0_raw[:, 512 * j + 256:512 * (j + 1)],
                                din["m0T"][:, 512 * j + 256:512 * (j + 1)])
                        else:
                            eng = nc.sync if j % 2 == 0 else nc.scalar
                            eng.dma_start(
                                m0_raw[:, 512 * j:512 * (j + 1)],
                                din["m0T"][:, 512 * j:512 * (j + 1)])
                        nc.vector.tensor_copy(
                            m0r[:, 512 * j:512 * (j + 1)],
                            m0_raw[:, 512 * j:512 * (j + 1)])
                    mg1t = load("mg1c", (128, 2))
                    mbe1t = load("mbe1c", (128, 2))
                    ma1t = load("ma1r", (128, 1))
                    mg2t, mbe2t = load("mg2c", (128, 1)), load("mbe2c",
                                                               (128, 1))
                    ma2t = load("ma2r", (128, 1))
                    mg3t, mbe3t = load("mg3c", (64, 1)), load("mbe3c",
                                                              (64, 1))
                    ma3t = load("ma3r", (64, 1))
                    bngt, bnbt = load("bngc", (128, 2)), load("bnbc",
                                                              (128, 2))
                    art = load("ar", (128, 1))
                    x0t = load("x0T", (64, BC), bf16)
                    id64t = load("id64", (64, 64), bf16)
                    sel32t = load("sel32", (32, 4), f32)
                    benc0t = load("benc0", (E, 256), bf16)
                    benc1t = load("benc1", (E, 256), bf16)
                    bdec0t = load("bdec0", (E, 256), bf16)
                    bdec1t = load("bdec1", (E, 64), bf16)
                    w2r = []
                    for k in range(2):
                        w2_raw = raw.tile([128, 128], f32, name="w2_raw",
                                          tag="w2_raw")
                        nc.sync.dma_start(w2_raw[:],
                                          din["mW2T"][128 * k:128 * (k + 1), :])
                        w2rk = sb.tile([128, 128], f32r, name=f"w2r_{k}")
                        nc.vector.tensor_copy(w2rk[:], w2_raw[:])
                        w2r.append(w2rk)
                    w3_raw = raw.tile([128, 64], f32)
                    nc.sync.dma_start(w3_raw[:], din["mW3T"][:])
                    w3r = sb.tile([128, 64], f32r)
                    nc.vector.tensor_copy(w3r[:], w3_raw[:])

                # ======== layer 1: 128 -> 256 (single matmul pass) ====
                st1 = sb.tile([128, 4 * NB], f32)  # (s=2, b=NB, c=2)
                y1 = [mbig.tile([128, B], f32, name=f"y1_{c}")
                      for c in range(2)]
                for bc in range(NB):
                    for c in range(2):
                        ps = mps.tile([128, 512], f32, name="mnet_ps")
                        nc.tensor.matmul(ps[:], w1r[:, 128 * c:128 * (c + 1)],
                                         m0r[:, 512 * bc:512 * (bc + 1)],
                                         start=True, stop=True)
                        col = 2 * bc + c
                        nc.vector.tensor_scalar(
                            out=y1[c][:, 512 * bc:512 * (bc + 1)], in0=ps[:],
                            scalar1=1.0, scalar2=0.0, op0=OP.mult,
                            op1=OP.add, accum_out=st1[:, col:col + 1])
                        sq_scr = scr.tile([128, 512], bf16, name="sqscr")
                        nc.scalar.activation(
                            sq_scr[:], ps[:], AF.Square,
                            accum_out=st1[:, 2 * NB + col:
                                          2 * NB + col + 1])
                st1r = sb.tile([128, 4], f32)
                nc.vector.tensor_reduce(
                    st1r[:].rearrange("p (s c) -> p s c", s=2),
                    st1[:].rearrange("p (s b c) -> p s c b", s=2, b=NB, c=2),
                    axis=AX.X, op=OP.add)
                s1, t1 = _bn_apply_params(nc, sb, st1r, 2, B, mg1t[:],
                                          mbe1t[:], "m1")
                h1 = [mbig.tile([128, B], f32r, name=f"h1_{c}")
                      for c in range(2)]
                for bc in range(0, NB, 4):
                    for c in range(2):
                        nc.scalar.activation(
                            h1[c][:, 512 * bc:512 * (bc + 4)],
                            y1[c][:, 512 * bc:512 * (bc + 4)],
                            AF.Prelu, bias=t1[:, c:c + 1],
                            scale=s1[:, c:c + 1], alpha=ma1t[:])

                # ======== layer 2: 256 -> 128 (single pass) ========
                st2 = sb.tile([128, 2 * NB], f32)
                y2 = mbig.tile([128, B], f32)
                for bc in range(NB):
                    ps = mps.tile([128, 512], f32, name="mnet_ps")
                    for k in range(2):
                        nc.tensor.matmul(ps[:], w2r[k][:],
                                         h1[k][:, 512 * bc:512 * (bc + 1)],
                                         start=(k == 0), stop=(k == 1))
                    nc.vector.tensor_scalar(
                        out=y2[:, 512 * bc:512 * (bc + 1)], in0=ps[:],
                        scalar1=1.0, scalar2=0.0, op0=OP.mult,
                        op1=OP.add, accum_out=st2[:, bc:bc + 1])
                    sq_scr = scr.tile([128, 512], bf16, name="sqscr")
                    nc.scalar.activation(sq_scr[:], ps[:], AF.Square,
                                         accum_out=st2[:, NB + bc:NB + bc + 1])
                st2r = sb.tile([128, 2], f32)
                nc.vector.tensor_reduce(
                    st2r[:].rearrange("p (s one) -> p s one", one=1),
                    st2[:].rearrange("p (s b) -> p s b", s=2),
                    axis=AX.X, op=OP.add)
                s2, t2 = _bn_apply_params(nc, sb, st2r, 1, B, mg2t[:],
                                          mbe2t[:], "m2")
                h2 = mbig.tile([128, B], f32r)
                for bc in range(NB):
                    nc.scalar.activation(h2[:, 512 * bc:512 * (bc + 1)],
                                         y2[:, 512 * bc:512 * (bc + 1)],
                                         AF.Prelu, bias=t2[:], scale=s2[:],
                                         alpha=ma2t[:])

                # ======== layer 3: 128 -> 64 (w, single pass) ========
                st3 = sb.tile([64, 2 * NB], f32)
                y3 = mbig.tile([64, B], f32)
                for bc in range(NB):
                    ps = maux.tile([64, 512], f32, name="y3ps", bufs=2)
                    nc.tensor.matmul(ps[:], w3r[:],
                                     h2[:, 512 * bc:512 * (bc + 1)],
                                     start=True, stop=True)
                    nc.vector.tensor_scalar(
                        out=y3[:, 512 * bc:512 * (bc + 1)], in0=ps[:],
                        scalar1=1.0, scalar2=0.0, op0=OP.mult,
                        op1=OP.add, accum_out=st3[:, bc:bc + 1])
                    sq3 = scr.tile([64, 512], bf16, name="sq3scr")
                    nc.scalar.activation(sq3[:], ps[:], AF.Square,
                                         accum_out=st3[:, NB + bc:NB + bc + 1])
                st3r = sb.tile([64, 2], f32)
                nc.vector.tensor_reduce(
                    st3r[:].rearrange("p (s one) -> p s one", one=1),
                    st3[:].rearrange("p (s b) -> p s b", s=2),
                    axis=AX.X, op=OP.add)
                s3, t3 = _bn_apply_params(nc, sb, st3r, 1, B, mg3t[:],
                                          mbe3t[:], "m3")
                w_full = mbig.tile([64, B], f32)
                wrs = sb.tile([64, 1], f32)
                nc.scalar.activation(w_full[:], y3[:], AF.Prelu,
                                     bias=t3[:], scale=s3[:],
                                     alpha=ma3t[:], accum_out=wrs[:])
                ones64 = sb.tile([64, 1], f32)
                nc.vector.memset(ones64[:], 1.0)
                wsum_ps = maux.tile([1, 1], f32, name="wsum_ps", bufs=1)
                nc.tensor.matmul(wsum_ps[:], wrs[:], ones64[:],
                                 start=True, stop=True)
                invw = sb.tile([1, 1], f32)
                nc.vector.reciprocal(invw[:], wsum_ps[:])
                ones1 = sb.tile([1, 64], f32)
                nc.vector.memset(ones1[:], 1.0)
                inv64_ps = maux.tile([64, 1], f32, name="inv64_ps", bufs=1)
                nc.tensor.matmul(inv64_ps[:], ones1[:], invw[:],
                                 start=True, stop=True)
                inv64 = sb.tile([64, 1], f32)
                nc.vector.tensor_copy(inv64[:], inv64_ps[:])
                nc.vector.tensor_scalar(out=wnt[:], in0=w_full[:, 0:512],
                                        scalar1=inv64[:], scalar2=None,
                                        op0=OP.mult)
                if DEBUG:
                    wn_dbg = sb.tile([64, 512], f32)
                    nc.vector.tensor_scalar(out=wn_dbg[:],
                                            in0=w_full[:, 0:512],
                                            scalar1=inv64[:], scalar2=None,
                                            op0=OP.mult)
                    nc.sync.dma_start(dbg_wn[:], wn_dbg[:])

            # ================= expert phase =================
            with tc.tile_pool(name="trep", bufs=1) as trep, \
                 tc.tile_pool(name="wtp", bufs=8) as wtp, \
                 tc.tile_pool(name="zsp", bufs=5) as zsp, \
                 tc.tile_pool(name="pex", bufs=1, space="PSUM") as pex:

                # T tiles via PE selector matmul + ACT copy (no DMA traffic)
                Tg = trep.tile([128, E * 512], bf16, bufs=1)
                Tg3 = Tg[:].rearrange("p (e b) -> p e b", e=E)
                for e in range(E):
                    tps = pex.tile([128, 512], f32, name="tps", tag="tps",
                                   bufs=2)
                    nc.tensor.matmul(
                        tps[:], id64t[:, e:e + 1].broadcast_to([64, 128]),
                        wnt[:], start=True, stop=True)
                    nc.any.tensor_copy(Tg[:, 512 * e:512 * (e + 1)],
                                       tps[:])

                def stat_allgather(stat, tag):
                    """stat [128, 128] (first 4 cols: sum0,sum1,sq0,sq1) ->
                    summed over cores [128, 4]."""
                    statT = sb.tile([32, 128], f32, name=f"statT_{tag}")
                    for bi in range(4):
                        nc.vector.transpose(
                            statT[0:32, 32 * bi:32 * (bi + 1)],
                            stat[32 * bi:32 * (bi + 1), 0:32])
                    sd = dram.tile([4, 128], f32, name=f"sd_{tag}")
                    gd = dram.tile([N_CORES * 4, 128], f32,
                                   addr_space="Shared", name=f"gd_{tag}")
                    nc.sync.dma_start(sd[:], statT[0:4, :])
                    nc.gpsimd.collective_compute(
                        "AllGather", OP.bypass,
                        replica_groups=[list(range(N_CORES))],
                        ins=[sd.opt()], outs=[gd.opt()])
                    gt32 = sb.tile([32, 128], f32, name=f"gt_{tag}")
                    nc.sync.dma_start(gt32[:], gd[:])
                    totp = pex.tile([128, 4], f32, name=f"totp_{tag}",
                                    tag="totp", bufs=2)
                    nc.tensor.matmul(totp[:], gt32[:], sel32t[:],
                                     start=True, stop=True)
                    tot = sb.tile([128, 4], f32, name=f"tot_{tag}")
                    nc.any.tensor_copy(tot[:], totp[:])
                    return tot

                def expert_layer(zin, kchunks, ochunks, wdram, bias_t, tag,
                                 kpart=128, m=128):
                    """zin: bf16 [kpart, kchunks*512] input tile.
                    Returns list of PSUM tiles [m, 512]."""
                    wfree = wdram.shape[2]
                    psums = [pex.tile([m, 512], f32, name=f"px_{tag}{o}",
                                      tag=f"px{o}", bufs=2)
                             for o in range(ochunks)]
                    zw = kchunks * 512
                    for g in range(E // 4):
                        # zs for 4 experts in one DVE op
                        zs = zsp.tile([kpart, 4 * zw], bf16,
                                      name=f"zs_{tag}", tag="zs")
                        t4 = Tg3[0:kpart, 4 * g:4 * (g + 1), :]
                        zs4 = zs[:].rearrange("p (e k b) -> p e k b", e=4,
                                              k=kchunks)
                        for k in range(kchunks):
                            zin3 = zin[:, 512 * k:512 * (k + 1)] \
                                .unsqueeze(1).broadcast_to([kpart, 4, 512])
                            nc.vector.tensor_tensor(
                                out=zs4[:, :, k, :], in0=zin3, in1=t4,
                                op=OP.mult)
                        # weights: one DMA per 2 experts
                        for h in range(2):
                            wt = wtp.tile([kpart, 2 * kchunks * wfree], bf16,
                                          name=f"wt_{tag}", tag="wt")
                            eng = nc.sync if (h == 0 or tag == "e0") \
                                else nc.scalar
                            e0 = 4 * g + 2 * h
                            if kchunks == 1:
                                eng.dma_start(
                                    wt[:].rearrange("p (e o) -> p e o", e=2),
                                    wdram[e0:e0 + 2].rearrange(
                                        "e p o -> p e o"))
                            else:
                                eng.dma_start(
                                    wt[:].rearrange("p (e k o) -> p e k o",
                                                    e=2, k=kchunks),
                                    wdram[e0:e0 + 2].rearrange(
                                        "e (k p) o -> p e k o", k=kchunks))
                            for j in range(2):
                                ee = 2 * h + j
                                for k in range(kchunks):
                                    for o in range(ochunks):
                                        nc.tensor.matmul(
                                            psums[o][:],
                                            wt[:, kchunks * wfree * j
                                               + wfree * k + 128 * o:
                                               kchunks * wfree * j
                                               + wfree * k + 128 * o + m],
                                            zs[:, zw * ee + 512 * k:
                                               zw * ee + 512 * (k + 1)],
                                            start=(g == 0 and h == 0
                                                   and j == 0 and k == 0),
                                            stop=False)
                    for o in range(ochunks):
                        nc.tensor.matmul(psums[o][:],
                                         bias_t[:, 128 * o:128 * o + m],
                                         wnt[:], start=False, stop=True)
                    return psums

                def bn_prelu(psums, tag):
                    stat = sb.tile([128, 32], f32, name=f"stat_{tag}")
                    nc.vector.memset(stat[:], 0.0)
                    for o in range(2):
                        nc.vector.tensor_reduce(stat[:, o:o + 1], psums[o][:],
                                                axis=AX.X, op=OP.add)
                        sq = scr.tile([128, 512], bf16, name="sqscr")
                        nc.scalar.activation(sq[:], psums[o][:], AF.Square,
                                             accum_out=stat[:, 2 + o:3 + o])
                    tot = stat_allgather(stat, tag)
                    if DEBUG and tag == "e0":
                        nc.sync.dma_start(dbg_tot[:], tot[:])
                    s_ap, t_ap = _bn_apply_params(nc, sb, tot, 2, B, bngt[:],
                                                  bnbt[:], tag,
                                                  prescaled=True)
                    z = sb.tile([128, 1024], bf16, name=f"z_{tag}")
                    for o in range(2):
                        nc.scalar.activation(z[:, 512 * o:512 * (o + 1)],
                                             psums[o][:], AF.Prelu,
                                             bias=t_ap[:, o:o + 1],
                                             scale=s_ap[:, o:o + 1],
                                             alpha=art[:])
                    return z

                ps = expert_layer(x0t[:], 1, 2, Wenc0T, benc0t, "e0",
                                  kpart=64)
                if DEBUG:
                    td = sb.tile([128, 512], f32)
                    nc.vector.tensor_copy(td[:], Tg[:, 0:512])
                    nc.sync.dma_start(dbg_T[:], td[:])
                    for o in range(2):
                        pd = sb.tile([128, 512], f32, name=f"pd{o}")
                        nc.vector.tensor_copy(pd[:], ps[o][:])
                        nc.sync.dma_start(dbg_ps[128 * o:128 * (o + 1), :],
                                          pd[:])
                z = bn_prelu(ps, "e0")
                if DEBUG:
                    for o in range(2):
                        zd = sb.tile([128, 512], f32, name=f"zd{o}")
                        nc.vector.tensor_copy(zd[:],
                                              z[:, 512 * o:512 * (o + 1)])
                        nc.sync.dma_start(dbg_z0[128 * o:128 * (o + 1), :],
                                          zd[:])
                ps = expert_layer(z[:], 2, 2, Wenc1T, benc1t, "e1")
                z = bn_prelu(ps, "e1")
                ps = expert_layer(z[:], 2, 2, Wdec0T, bdec0t, "d0")
                z = bn_prelu(ps, "d0")
                ps = expert_layer(z[:], 2, 1, Wdec1T, bdec1t, "d1", m=64)
                out_sb = sb.tile([64, 512], f32)
                nc.scalar.copy(out_sb[:], ps[0][:])
                nc.sync.dma_start(out[:], out_sb[:])

    nc.compile()
    return nc


def _prep_inputs(inputs):
    """Host-side marshalling: returns per-core in_maps."""
    bf = ml_dtypes.bfloat16
    f = np.float32
    m0 = np.asarray(inputs["m0"], f)
    x0 = np.asarray(inputs["x0"], f)
    m0T_full = np.ascontiguousarray(m0.T)           # [128, 4096]
    x0T_full = np.ascontiguousarray(x0.T)           # [64, 4096]

    def chunk2(v, nch):  # [F] -> [F//nch, nch] chunk-major
        v = np.asarray(v, f)
        p = v.shape[0] // nch
        return np.ascontiguousarray(v.reshape(nch, p).T)

    def rep(v, p):
        return np.full((p, 1), np.asarray(v, f).reshape(-1)[0], f)

    shared = {
        "mW1T": np.ascontiguousarray(np.asarray(inputs["mW1"], f).T),
        "mW2T": np.ascontiguousarray(np.asarray(inputs["mW2"], f).T),
        "mW3T": np.ascontiguousarray(np.asarray(inputs["mW3"], f).T),
        "mg1c": chunk2(inputs["mg1"], 2), "mbe1c": chunk2(inputs["mbe1"], 2),
        "ma1r": rep(inputs["ma1"], 128),
        "mg2c": chunk2(inputs["mg2"], 1), "mbe2c": chunk2(inputs["mbe2"], 1),
        "ma2r": rep(inputs["ma2"], 128),
        "mg3c": chunk2(inputs["mg3"], 1), "mbe3c": chunk2(inputs["mbe3"], 1),
        "ma3r": rep(inputs["ma3"], 64),
        "bngc": chunk2(inputs["bng"], 2), "bnbc": chunk2(inputs["bnb"], 2),
        "ar": rep(inputs["a"], 128),
        "Wenc0T": np.ascontiguousarray(
            np.asarray(inputs["Wenc0"], f).transpose(0, 2, 1)).astype(bf),
        "Wenc1T": np.ascontiguousarray(
            np.asarray(inputs["Wenc1"], f).transpose(0, 2, 1)).astype(bf),
        "Wdec0T": np.ascontiguousarray(
            np.asarray(inputs["Wdec0"], f).transpose(0, 2, 1)).astype(bf),
        "Wdec1T": np.ascontiguousarray(
            np.asarray(inputs["Wdec1"], f).transpose(0, 2, 1)).astype(bf),
        "id64": np.eye(64, dtype=f).astype(bf),
        "sel32": np.tile(np.eye(4, dtype=f) / 4096.0, (8, 1)),
        "benc0": np.asarray(inputs["benc0"], f).astype(bf),
        "benc1": np.asarray(inputs["benc1"], f).astype(bf),
        "bdec0": np.asarray(inputs["bdec0"], f).astype(bf),
        "bdec1": np.asarray(inputs["bdec1"], f).astype(bf),
    }
    in_maps = []
    for i in range(N_CORES):
        r = BC * i
        m0T_rot = np.ascontiguousarray(
            np.concatenate([m0T_full[:, r:], m0T_full[:, :r]], axis=1))
        x0T_sl = np.ascontiguousarray(x0T_full[:, r:r + BC]).astype(bf)
        m = dict(shared)
        m["m0T"] = m0T_rot
        m["x0T"] = x0T_sl
        in_maps.append(m)
    return in_maps


def kernel(**inputs) -> np.ndarray:
    if "nc" not in _cache:
        _cache["nc"] = _build()
    nc = _cache["nc"]
    in_maps = _prep_inputs(inputs)
    res = run_bass_kernel_spmd(nc, in_maps, core_ids=list(range(N_CORES)))
    y = np.empty((B, 64), np.float32)
    for i in range(N_CORES):
        y[BC * i:BC * (i + 1), :] = res.results[i]["out"].T
    return y
